# revision 13
# baseline (speedup 1.0000x reference)
"""Trainium2 Bass kernel for nn_NodeBlock (GNN message passing), v4.

Pipeline: segment_sum of edge features onto destination nodes, concat with
node features, 3-layer MLP, LayerNorm.

Layout: nodes are packed into 800 blocks of 128 (LPT on overflow degree),
blocks dealt to 8 cores.  Every node owns a FIXED span of 16 "main" edge
slots (4 groups of G=4); edges beyond 16 per node go to a per-block
remainder region (up to 256 slots).  Edges are fp16 (~7e-4 rel err vs the
2e-2 gate).

Segment sum is two-stage with constant matrices: per block, 16 main edge
tiles are pre-reduced on the PE with a fixed G=4 grouping matrix R
(col-tiled 4x: 32-col matmuls at tile_position (0,32i) -> one [128,512]
PSUM of per-group sums); because group->node is fixed, the scatter of the
512 groups onto 128 nodes is 4 matmuls against a CONSTANT selection matrix
S_q (no per-tile DVE work).  Only the <=256 remainder edges per block use
DVE one-hot (is_equal vs iota) scatter.

Four blocks form a superblock (512 nodes) with a fused fp16 MLP + LayerNorm
epilogue: column stats via stationary matmuls against a 1/128 ones column,
PE transposes to node-major, normalize via DVE tensor_scalar, gamma/beta
TTs.  Output is fp16, node-permuted; the host undoes the permutation.
"""

import sys

sys.path.insert(0, "/opt/trn_rl_repo")

import numpy as np

N_CORES = 8
NUM_NODES = 100000
D = 128            # node/edge feature dim
P = 128            # partitions
BLK = 128          # nodes per block
G = 4              # edges per pre-reduction group
CAP = 16           # main edge slots per node (4 groups)
KMAIN = 16         # main edge tiles per block (= BLK*CAP/128, 4 quads)
KREM = 2           # remainder edge tiles per block (direct one-hot)
KTOT = KMAIN + KREM
BLOCKS_PER_CORE = 100
SB = 4             # blocks per superblock
SBLOCKS = BLOCKS_PER_CORE // SB          # 25
NODES_PER_CORE = BLK * BLOCKS_PER_CORE   # 12800
TOTAL_BLOCKS = N_CORES * BLOCKS_PER_CORE  # 800
EPS = 1e-5

_nc_cache = {}
last_run_info = {}

TUNE = {"ebufs": 2, "ohbufs": 8, "sbufs": 3, "agbufs": 2, "mlpbufs": 1,
        "pqbufs": 2, "gam_engine": "dve", "beta_engine": "dve",
        "sq_engine": "dve", "grp_dve": 2, "s1_first": False,
        "only": None}


def _build_nc(kb, loop_iters=None):
    """kb is kept for test.py compatibility; v4 uses fixed KTOT tiles."""
    import contextlib
    import concourse.bacc as bacc
    import concourse.tile as tile
    import concourse.mybir as mybir

    dt = mybir.dt
    f32 = dt.float32
    f16 = dt.float16
    tot_e = BLOCKS_PER_CORE * KTOT * 128

    nc = bacc.Bacc("TRN2", target_bir_lowering=False, debug=False,
                   name="nodeblock")

    edges = nc.dram_tensor("edges", [P, tot_e], f16, kind="ExternalInput")
    colf32 = nc.dram_tensor("colf32", [P, BLOCKS_PER_CORE * KREM],
                            f32, kind="ExternalInput")
    natT = nc.dram_tensor("natT", [P, NODES_PER_CORE], f16,
                          kind="ExternalInput")
    iota = nc.dram_tensor("iota", [P, 128], f16, kind="ExternalInput")
    w_in = {}
    for nm in ["w0a", "w0b", "w1", "w2", "ident"]:
        w_in[nm] = nc.dram_tensor(nm, [128, 128], f16, kind="ExternalInput")
    for nm in ["b0", "b1", "b2"]:
        w_in[nm] = nc.dram_tensor(nm, [128, 1], f32, kind="ExternalInput")
    w_in["rmat"] = nc.dram_tensor("rmat", [128, 32], f16,
                                  kind="ExternalInput")
    w_in["smat"] = nc.dram_tensor("smat", [128, SB, 128], f16,
                                  kind="ExternalInput")
    w_in["gam"] = nc.dram_tensor("gam", [128, SB, 128], f16,
                                 kind="ExternalInput")
    w_in["bet"] = nc.dram_tensor("bet", [128, SB, 128], f16,
                                 kind="ExternalInput")
    out = nc.dram_tensor("out", [SBLOCKS, P, SB, 128], f16,
                         kind="ExternalOutput")

    with tile.TileContext(nc) as tc:
        with (
            tc.tile_pool(name="const", bufs=1) as cpool,
            tc.tile_pool(name="edge", bufs=TUNE["ebufs"]) as epool,
            tc.tile_pool(name="oh", bufs=TUNE["ohbufs"]) as ohpool,
            tc.tile_pool(name="small", bufs=TUNE["sbufs"]) as spool,
            tc.tile_pool(name="psag", bufs=TUNE["agbufs"],
                         space="PSUM") as psag,
            tc.tile_pool(name="psmlp", bufs=TUNE["mlpbufs"],
                         space="PSUM") as psmlp,
            tc.tile_pool(name="pspq", bufs=TUNE["pqbufs"],
                         space="PSUM") as pspq,
            tc.tile_pool(name="psaux", bufs=1, space="PSUM") as psaux,
        ):
            cdma = nc.scalar
            colf_s = cpool.tile([P, BLOCKS_PER_CORE * KREM], f32,
                                tag="colf32", name="colf32")
            cdma.dma_start(out=colf_s[:], in_=colf32[:])
            natT_s = cpool.tile([P, NODES_PER_CORE], f16, tag="natT",
                                name="natT")
            cdma.dma_start(out=natT_s[:], in_=natT[:])
            iota_s = cpool.tile([P, 128], f16, tag="iota", name="iota")
            cdma.dma_start(out=iota_s[:], in_=iota[:])
            consts = {}
            for nm, t in w_in.items():
                dtt = f32 if nm in ("b0", "b1", "b2") else f16
                consts[nm] = cpool.tile(list(t.shape), dtt, tag=nm, name=nm)
                cdma.dma_start(out=consts[nm][:], in_=t[:])
            onesc = cpool.tile([P, 1], f16, tag="onesc", name="onesc")
            nc.vector.memset(onesc[:], 1.0 / 128.0)
            epst = cpool.tile([P, 1], f32, tag="eps", name="eps")
            nc.vector.memset(epst[:], EPS)

            loop_cm = (tc.For_i(0, loop_iters, 1) if loop_iters
                       else contextlib.nullcontext())
            with loop_cm:
                _emit(nc, tc, epool, ohpool, spool, psag, psmlp, pspq,
                      psaux, colf_s, natT_s, iota_s, consts, onesc, epst,
                      edges, out, mybir)
    nc.finalize()
    return nc


def _emit(nc, tc, epool, ohpool, spool, psag, psmlp, pspq, psaux, colf_s,
          natT_s, iota_s, consts, onesc, epst, edges, out, mybir):
    dt = mybir.dt
    f32 = dt.float32
    f16 = dt.float16
    Alu = mybir.AluOpType
    Act = mybir.ActivationFunctionType
    only = TUNE["only"]
    edma = nc.sync
    odma = nc.scalar
    R = consts["rmat"]
    S = consts["smat"]
    sb_e = SB * KTOT * 128          # edge elems per superblock per partition

    for s in range(SBLOCKS):
        if only in (None, "dma", "agg", "s1"):
            eblk = epool.tile([P, sb_e], f16, tag="eblk", name="eblk")
            edma.dma_start(out=eblk[:], in_=edges[:, s * sb_e:(s + 1) * sb_e])
        if only == "dma":
            continue

        if only == "dve":
            for c in range(SB * KREM):
                oh = ohpool.tile([P, 128], f16, tag="oh", name="oh")
                nc.vector.tensor_scalar(
                    out=oh[:], in0=iota_s[:],
                    scalar1=colf_s[:, s * SB * KREM + c:
                                   s * SB * KREM + c + 1],
                    scalar2=None, op0=Alu.is_equal)
            continue

        pag = psag.tile([P, SB * 128], f32, tag="ag", name="ag",
                        bufs=TUNE["agbufs"])
        pqs = []
        if TUNE["s1_first"]:
            for b4 in range(SB):
                ebase = (b4 * KTOT) * 128
                pq = pspq.tile([P, 512], f32, tag="pq", name="pq",
                               bufs=TUNE["pqbufs"])
                pqs.append(pq)
                for t in range(KMAIN):
                    i, q = t % 4, t // 4
                    nc.tensor.matmul(
                        out=pq[32 * i:32 * i + 32, 128 * q:128 * q + 128],
                        lhsT=R[:],
                        rhs=eblk[:, ebase + t * 128:ebase + (t + 1) * 128],
                        tile_position=(0, 32 * i), start=True, stop=True)
        for b4 in range(SB):
            ebase = (b4 * KTOT) * 128
            if TUNE["s1_first"]:
                pq = pqs[b4]
            else:
                # stage 1: 16 main tiles -> [128,512] groups (col-tiled 4x)
                pq = pspq.tile([P, 512], f32, tag="pq", name="pq",
                               bufs=TUNE["pqbufs"])
                for t in range(KMAIN):
                    i, q = t % 4, t // 4
                    nc.tensor.matmul(
                        out=pq[32 * i:32 * i + 32, 128 * q:128 * q + 128],
                        lhsT=R[:],
                        rhs=eblk[:, ebase + t * 128:ebase + (t + 1) * 128],
                        tile_position=(0, 32 * i), start=True, stop=True)
            if only == "s1":
                continue
            grp = spool.tile([P, 512], f16, tag="grp", name="grp")
            geng = nc.vector if b4 < TUNE["grp_dve"] else nc.scalar
            if geng is nc.vector:
                nc.vector.tensor_copy(grp[:], pq[:])
            else:
                nc.scalar.copy(grp[:], pq[:])
            # stage 2: 4 constant-scatter matmuls + 2 remainder edge tiles
            cbase = s * SB * KREM + b4 * KREM
            for q in range(SB):
                nc.tensor.matmul(out=pag[:, b4 * 128:(b4 + 1) * 128],
                                 lhsT=grp[:, q * 128:(q + 1) * 128],
                                 rhs=S[:, q, :], start=(q == 0), stop=False)
            for r in range(KREM):
                oh = ohpool.tile([P, 128], f16, tag="oh", name="oh")
                nc.vector.tensor_scalar(
                    out=oh[:], in0=iota_s[:],
                    scalar1=colf_s[:, cbase + r:cbase + r + 1],
                    scalar2=None, op0=Alu.is_equal)
                nc.tensor.matmul(
                    out=pag[:, b4 * 128:(b4 + 1) * 128],
                    lhsT=eblk[:, ebase + (KMAIN + r) * 128:
                              ebase + (KMAIN + r + 1) * 128],
                    rhs=oh[:], start=False, stop=(r == KREM - 1))
        if only in ("agg", "s1"):
            continue

        aggrT = spool.tile([P, SB * 128], f16, tag="aggrT", name="aggrT")
        nc.scalar.copy(aggrT[:], pag[:])

        # MLP (fp16 weights, fp32 PSUM accumulate)
        ph1 = psmlp.tile([P, SB * 128], f32, tag="mlp", name="mlp")
        nc.tensor.matmul(out=ph1[:], lhsT=consts["w0a"][:],
                         rhs=natT_s[:, s * SB * 128:(s + 1) * SB * 128],
                         start=True, stop=False)
        nc.tensor.matmul(out=ph1[:], lhsT=consts["w0b"][:], rhs=aggrT[:],
                         start=False, stop=True)
        h1 = spool.tile([P, SB * 128], f16, tag="h1", name="h1")
        nc.scalar.activation(h1[:], ph1[:], Act.Relu, bias=consts["b0"][:])

        ph2 = psmlp.tile([P, SB * 128], f32, tag="mlp", name="mlp")
        nc.tensor.matmul(out=ph2[:], lhsT=consts["w1"][:], rhs=h1[:],
                         start=True, stop=True)
        h2 = spool.tile([P, SB * 128], f16, tag="h2", name="h2")
        nc.scalar.activation(h2[:], ph2[:], Act.Relu, bias=consts["b1"][:])

        ph3 = psmlp.tile([P, SB * 128], f32, tag="mlp", name="mlp")
        nc.tensor.matmul(out=ph3[:], lhsT=consts["w2"][:], rhs=h2[:],
                         start=True, stop=True)
        h3T = spool.tile([P, SB * 128], f16, tag="h3T", name="h3T")
        nc.scalar.activation(h3T[:], ph3[:], Act.Identity,
                             bias=consts["b2"][:])
        sq = spool.tile([P, SB * 128], f16, tag="sq", name="sq")
        if TUNE["sq_engine"] == "dve":
            nc.vector.tensor_tensor(out=sq[:], in0=h3T[:], in1=h3T[:],
                                    op=Alu.mult)
        else:
            nc.scalar.activation(sq[:], h3T[:], Act.Square)

        # column stats: mu and E[x^2] per node into one aux PSUM bank
        paux = psag.tile([P, 2 * SB], f32, tag="py", name="aux", bufs=2)
        for b4 in range(SB):
            nc.tensor.matmul(out=paux[:, b4:b4 + 1],
                             lhsT=h3T[:, b4 * 128:(b4 + 1) * 128],
                             rhs=onesc[:], start=True, stop=True)
        for b4 in range(SB):
            nc.tensor.matmul(out=paux[:, SB + b4:SB + b4 + 1],
                             lhsT=sq[:, b4 * 128:(b4 + 1) * 128],
                             rhs=onesc[:], start=True, stop=True)
        mu_sb = spool.tile([P, SB], f32, tag="mu", name="mu")
        nc.scalar.copy(mu_sb[:], paux[:, 0:SB])
        musq = spool.tile([P, SB], f32, tag="musq", name="musq")
        nc.vector.tensor_tensor(out=musq[:], in0=mu_sb[:], in1=mu_sb[:],
                                op=Alu.mult)
        var = spool.tile([P, SB], f32, tag="var", name="var")
        nc.vector.tensor_tensor(out=var[:], in0=paux[:, SB:2 * SB],
                                in1=musq[:], op=Alu.subtract)
        std = spool.tile([P, SB], f32, tag="std", name="std")
        nc.scalar.activation(std[:], var[:], Act.Sqrt, bias=epst[:])
        rstd = spool.tile([P, SB], f32, tag="rstd", name="rstd")
        nc.vector.reciprocal(rstd[:], std[:])

        pyt = psag.tile([P, SB, 128], f32, tag="py", name="py", bufs=2)
        for b4 in range(SB):
            nc.tensor.matmul(out=pyt[:, b4, :],
                             lhsT=h3T[:, b4 * 128:(b4 + 1) * 128],
                             rhs=consts["ident"][:], start=True, stop=True)
        xn = spool.tile([P, SB, 128], f16, tag="xn", name="xn")
        for b4 in range(SB):
            nc.vector.tensor_scalar(
                out=xn[:, b4, :], in0=pyt[:, b4, :],
                scalar1=mu_sb[:, b4:b4 + 1], scalar2=rstd[:, b4:b4 + 1],
                op0=Alu.subtract, op1=Alu.mult)
        geng = nc.gpsimd if TUNE["gam_engine"] == "gp" else nc.vector
        beng = nc.gpsimd if TUNE["beta_engine"] == "gp" else nc.vector
        yg = spool.tile([P, SB, 128], f16, tag="yg", name="yg")
        geng.tensor_tensor(out=yg[:], in0=xn[:], in1=consts["gam"][:],
                           op=Alu.mult)
        yo = spool.tile([P, SB, 128], f16, tag="yo", name="yo")
        beng.tensor_tensor(out=yo[:], in0=yg[:], in1=consts["bet"][:],
                           op=Alu.add)
        odma.dma_start(out=out[s], in_=yo[:])


def _prepare_shards(node_attr, edge_attr, col):
    """Fixed 16-slot-per-node main region + LPT on overflow for remainder."""
    import heapq

    deg = np.bincount(col, minlength=NUM_NODES).astype(np.int64)
    over = np.maximum(deg - CAP, 0)
    order_nodes = np.argsort(-over, kind="stable")
    heap = [(0, 0, b) for b in range(TOTAL_BLOCKS)]
    heapq.heapify(heap)
    block_nodes = [[] for _ in range(TOTAL_BLOCKS)]
    for nd in order_nodes:
        d = int(over[nd])
        s, cnt, b = heapq.heappop(heap)
        block_nodes[b].append(int(nd))
        if cnt + 1 < BLK:
            heapq.heappush(heap, (s + d, cnt + 1, b))
    rem_max = max(sum(int(over[nd]) for nd in bn) for bn in block_nodes)
    assert rem_max <= KREM * 128, rem_max

    pos = np.full(NUM_NODES, -1, dtype=np.int64)      # old -> new node id
    natp = np.zeros((TOTAL_BLOCKS * BLK, D), np.float16)
    for b, bn in enumerate(block_nodes):
        ids = np.asarray(bn, dtype=np.int64)
        pos[ids] = b * BLK + np.arange(len(ids))
        natp[b * BLK:b * BLK + len(ids)] = node_attr[ids].astype(np.float16)
    assert (pos >= 0).all()

    # per-edge slot assignment
    order = np.argsort(col, kind="stable")           # edges grouped per node
    cs = col[order]
    within = np.arange(col.shape[0], dtype=np.int64)
    starts = np.zeros(NUM_NODES + 1, np.int64)
    starts[1:] = np.cumsum(deg)
    within = within - starts[cs]                     # rank within node
    npos = pos[cs]
    blk = npos >> 7
    loc = npos & 127

    main_mask = within < CAP
    slot = np.empty(col.shape[0], dtype=np.int64)
    slot[main_mask] = (blk[main_mask] * KTOT * 128 + loc[main_mask] * CAP +
                       within[main_mask])
    # overflow edges: sequential within their block's remainder region
    om = ~main_mask
    oblk = blk[om]
    oord = np.argsort(oblk, kind="stable")
    ocnt = np.bincount(oblk, minlength=TOTAL_BLOCKS)
    ostart = np.zeros(TOTAL_BLOCKS + 1, np.int64)
    ostart[1:] = np.cumsum(ocnt)
    opos_in_blk = np.arange(om.sum(), dtype=np.int64) - ostart[oblk[oord]]
    oslot = np.empty(om.sum(), dtype=np.int64)
    oslot[oord] = (oblk[oord] * KTOT * 128 + KMAIN * 128 + opos_in_blk)
    slot[om] = oslot

    ea16 = edge_attr.astype(np.float16)
    slots_per_core = BLOCKS_PER_CORE * KTOT * 128
    edges_by_core = []
    colf_by_core = []
    natT_by_core = []
    blk_of = slot // (KTOT * 128)
    off_of = slot % (KTOT * 128)
    loc_f = loc.astype(np.float32)
    for c in range(N_CORES):
        sel = (blk_of >= c * BLOCKS_PER_CORE) & \
              (blk_of < (c + 1) * BLOCKS_PER_CORE)
        lblk = blk_of[sel] - c * BLOCKS_PER_CORE
        lslot = lblk * (KTOT * 128) + off_of[sel]
        ebuf = np.zeros((slots_per_core, D), np.float16)
        ebuf[lslot] = ea16[order[sel]]
        earr = np.ascontiguousarray(
            ebuf.reshape(BLOCKS_PER_CORE * KTOT, 128, D)
            .transpose(1, 0, 2).reshape(P, slots_per_core))
        edges_by_core.append(earr)
        cf = np.full((BLOCKS_PER_CORE, KREM, 128), -1.0, np.float32)
        rm = off_of[sel] >= KMAIN * 128
        roff = off_of[sel][rm] - KMAIN * 128
        cf[lblk[rm], roff // 128, roff % 128] = loc_f[sel][rm]
        carr = np.ascontiguousarray(
            cf.reshape(BLOCKS_PER_CORE * KREM, 128).T)
        colf_by_core.append(carr)
        natT_by_core.append(np.ascontiguousarray(
            natp[c * NODES_PER_CORE:(c + 1) * NODES_PER_CORE].T))
    kb = (KTOT,) * BLOCKS_PER_CORE
    return kb, edges_by_core, colf_by_core, natT_by_core, pos


def kernel(node_attr, edge_attr, edge_index, W0, b0, W1, b1, W2, b2,
           ln_g, ln_b):
    from concourse import bass_utils

    node_attr = np.ascontiguousarray(np.asarray(node_attr, dtype=np.float32))
    edge_attr = np.ascontiguousarray(np.asarray(edge_attr, dtype=np.float32))
    col = np.asarray(edge_index)[1].astype(np.int64)
    W0 = np.asarray(W0, dtype=np.float32)
    W1 = np.asarray(W1, dtype=np.float32)
    W2 = np.asarray(W2, dtype=np.float32)
    b0v = np.asarray(b0, np.float32).reshape(128, 1).copy()
    b1v = np.asarray(b1, np.float32).reshape(128, 1).copy()
    b2v = np.asarray(b2, np.float32).reshape(128, 1).copy()
    gam = np.ascontiguousarray(
        np.broadcast_to(np.asarray(ln_g, np.float32).reshape(1, 1, 128),
                        (128, SB, 128)).astype(np.float16))
    bet = np.ascontiguousarray(
        np.broadcast_to(np.asarray(ln_b, np.float32).reshape(1, 1, 128),
                        (128, SB, 128)).astype(np.float16))

    kb, edges_by_core, colf_by_core, natT_by_core, pos = _prepare_shards(
        node_attr, edge_attr, col)

    iota_rep = np.ascontiguousarray(
        np.broadcast_to(np.arange(128, dtype=np.float16), (P, 128)))
    rmat = np.zeros((128, 32), np.float16)
    rmat[np.arange(128), np.arange(128) // G] = 1.0
    # S_q[p, j] = 1 iff node j owns group at partition p of quad-column q:
    #   j = 32 q + 8 (p//32) + (p%32)//4
    smat = np.zeros((128, SB, 128), np.float16)
    pidx = np.arange(128)
    for q in range(SB):
        smat[pidx, q, 32 * q + 8 * (pidx // 32) + (pidx % 32) // 4] = 1.0
    ident = np.eye(128, dtype=np.float16)

    if kb not in _nc_cache:
        _nc_cache[kb] = _build_nc(kb)
    nc = _nc_cache[kb]

    shared = {"iota": iota_rep, "rmat": rmat, "smat": smat, "ident": ident,
              "w0a": np.ascontiguousarray(W0[:128].astype(np.float16)),
              "w0b": np.ascontiguousarray(W0[128:].astype(np.float16)),
              "w1": np.ascontiguousarray(W1.astype(np.float16)),
              "w2": np.ascontiguousarray(W2.astype(np.float16)),
              "gam": gam, "bet": bet, "b0": b0v, "b1": b1v, "b2": b2v}
    in_maps = []
    for c in range(N_CORES):
        m = {"edges": edges_by_core[c], "colf32": colf_by_core[c],
             "natT": natT_by_core[c]}
        m.update(shared)
        in_maps.append(m)

    res = bass_utils.run_bass_kernel_spmd(nc, in_maps,
                                          core_ids=list(range(N_CORES)))
    last_run_info["results"] = res
    last_run_info["nc"] = nc
    last_run_info["in_maps"] = in_maps
    last_run_info["kb"] = kb
    last_run_info["pos"] = pos

    rows = np.concatenate(
        [res.results[c]["out"].reshape(SBLOCKS, P, SB, 128)
         .transpose(0, 2, 1, 3).reshape(NODES_PER_CORE, D)
         for c in range(N_CORES)], axis=0)
    return rows[pos].astype(np.float32)


if __name__ == "__main__":
    pass


# revision 14
# speedup vs baseline: 1.0134x; 1.0134x over previous
"""Trainium2 Bass kernel for nn_NodeBlock (GNN message passing), v4.

Pipeline: segment_sum of edge features onto destination nodes, concat with
node features, 3-layer MLP, LayerNorm.

Layout: nodes are packed into 800 blocks of 128 (LPT on overflow degree),
blocks dealt to 8 cores.  Every node owns a FIXED span of 16 "main" edge
slots (4 groups of G=4); edges beyond 16 per node go to a per-block
remainder region (up to 256 slots).  Edges are fp16 (~7e-4 rel err vs the
2e-2 gate).

Segment sum is two-stage with constant matrices: per block, 16 main edge
tiles are pre-reduced on the PE with a fixed G=4 grouping matrix R
(col-tiled 4x: 32-col matmuls at tile_position (0,32i) -> one [128,512]
PSUM of per-group sums); because group->node is fixed, the scatter of the
512 groups onto 128 nodes is 4 matmuls against a CONSTANT selection matrix
S_q (no per-tile DVE work).  Only the <=256 remainder edges per block use
DVE one-hot (is_equal vs iota) scatter.

Four blocks form a superblock (512 nodes) with a fused fp16 MLP + LayerNorm
epilogue: column stats via stationary matmuls against a 1/128 ones column,
PE transposes to node-major, normalize via DVE tensor_scalar, gamma/beta
TTs.  Output is fp16, node-permuted; the host undoes the permutation.
"""

import sys

sys.path.insert(0, "/opt/trn_rl_repo")

import numpy as np

N_CORES = 8
NUM_NODES = 100000
D = 128            # node/edge feature dim
P = 128            # partitions
BLK = 128          # nodes per block
G = 4              # edges per pre-reduction group
CAP = 16           # main edge slots per node (4 groups)
KMAIN = 16         # main edge tiles per block (= BLK*CAP/128, 4 quads)
KREM = 2           # remainder edge tiles per block (direct one-hot)
KTOT = KMAIN + KREM
BLOCKS_PER_CORE = 100
SB = 4             # blocks per superblock
SBLOCKS = BLOCKS_PER_CORE // SB          # 25
NODES_PER_CORE = BLK * BLOCKS_PER_CORE   # 12800
TOTAL_BLOCKS = N_CORES * BLOCKS_PER_CORE  # 800
EPS = 1e-5

_nc_cache = {}
last_run_info = {}

TUNE = {"ebufs": 2, "ohbufs": 8, "sbufs": 3, "agbufs": 2, "mlpbufs": 1,
        "pqbufs": 2, "gam_engine": "dve", "beta_engine": "dve",
        "sq_engine": "dve", "grp_dve": 2, "s1_first": False,
        "only": None}


def _build_nc(kb, loop_iters=None):
    """kb is kept for test.py compatibility; v4 uses fixed KTOT tiles."""
    import contextlib
    import concourse.bacc as bacc
    import concourse.tile as tile
    import concourse.mybir as mybir

    dt = mybir.dt
    f32 = dt.float32
    f16 = dt.float16
    tot_e = BLOCKS_PER_CORE * KTOT * 128

    nc = bacc.Bacc("TRN2", target_bir_lowering=False, debug=False,
                   name="nodeblock")

    edges = nc.dram_tensor("edges", [P, tot_e], f16, kind="ExternalInput")
    colf32 = nc.dram_tensor("colf32", [P, BLOCKS_PER_CORE * KREM],
                            f32, kind="ExternalInput")
    natT = nc.dram_tensor("natT", [P, NODES_PER_CORE], f16,
                          kind="ExternalInput")
    iota = nc.dram_tensor("iota", [P, 128], f16, kind="ExternalInput")
    w_in = {}
    for nm in ["w0a", "w0b", "w1", "w2", "ident"]:
        w_in[nm] = nc.dram_tensor(nm, [128, 128], f16, kind="ExternalInput")
    for nm in ["b0", "b1", "b2"]:
        w_in[nm] = nc.dram_tensor(nm, [128, 1], f32, kind="ExternalInput")
    w_in["rmat"] = nc.dram_tensor("rmat", [128, 32], f16,
                                  kind="ExternalInput")
    w_in["smat"] = nc.dram_tensor("smat", [128, SB, 128], f16,
                                  kind="ExternalInput")
    w_in["gam"] = nc.dram_tensor("gam", [128, SB, 128], f16,
                                 kind="ExternalInput")
    w_in["bet"] = nc.dram_tensor("bet", [128, SB, 128], f16,
                                 kind="ExternalInput")
    out = nc.dram_tensor("out", [SBLOCKS, P, SB, 128], f16,
                         kind="ExternalOutput")

    with tile.TileContext(nc) as tc:
        with (
            tc.tile_pool(name="const", bufs=1) as cpool,
            tc.tile_pool(name="edge", bufs=TUNE["ebufs"]) as epool,
            tc.tile_pool(name="oh", bufs=TUNE["ohbufs"]) as ohpool,
            tc.tile_pool(name="small", bufs=TUNE["sbufs"]) as spool,
            tc.tile_pool(name="psag", bufs=TUNE["agbufs"],
                         space="PSUM") as psag,
            tc.tile_pool(name="psmlp", bufs=TUNE["mlpbufs"],
                         space="PSUM") as psmlp,
            tc.tile_pool(name="pspq", bufs=TUNE["pqbufs"],
                         space="PSUM") as pspq,
            tc.tile_pool(name="psaux", bufs=1, space="PSUM") as psaux,
        ):
            cdma = nc.scalar
            colf_s = cpool.tile([P, BLOCKS_PER_CORE * KREM], f32,
                                tag="colf32", name="colf32")
            cdma.dma_start(out=colf_s[:], in_=colf32[:])
            natT_s = cpool.tile([P, NODES_PER_CORE], f16, tag="natT",
                                name="natT")
            cdma.dma_start(out=natT_s[:], in_=natT[:])
            iota_s = cpool.tile([P, 128], f16, tag="iota", name="iota")
            cdma.dma_start(out=iota_s[:], in_=iota[:])
            consts = {}
            for nm, t in w_in.items():
                dtt = f32 if nm in ("b0", "b1", "b2") else f16
                consts[nm] = cpool.tile(list(t.shape), dtt, tag=nm, name=nm)
                cdma.dma_start(out=consts[nm][:], in_=t[:])
            onesc = cpool.tile([P, 1], f16, tag="onesc", name="onesc")
            nc.vector.memset(onesc[:], 1.0 / 128.0)
            epst = cpool.tile([P, 1], f32, tag="eps", name="eps")
            nc.vector.memset(epst[:], EPS)

            loop_cm = (tc.For_i(0, loop_iters, 1) if loop_iters
                       else contextlib.nullcontext())
            with loop_cm:
                _emit(nc, tc, epool, ohpool, spool, psag, psmlp, pspq,
                      psaux, colf_s, natT_s, iota_s, consts, onesc, epst,
                      edges, out, mybir)
    nc.finalize()
    return nc


def _emit(nc, tc, epool, ohpool, spool, psag, psmlp, pspq, psaux, colf_s,
          natT_s, iota_s, consts, onesc, epst, edges, out, mybir):
    dt = mybir.dt
    f32 = dt.float32
    f16 = dt.float16
    Alu = mybir.AluOpType
    Act = mybir.ActivationFunctionType
    only = TUNE["only"]
    edma = nc.sync
    odma = nc.scalar
    R = consts["rmat"]
    S = consts["smat"]
    sb_e = SB * KTOT * 128          # edge elems per superblock per partition

    for s in range(SBLOCKS):
        if only in (None, "dma", "agg", "s1"):
            eblk = epool.tile([P, sb_e], f16, tag="eblk", name="eblk")
            edma.dma_start(out=eblk[:], in_=edges[:, s * sb_e:(s + 1) * sb_e])
        if only == "dma":
            continue

        if only == "dve":
            for c in range(SB * KREM):
                oh = ohpool.tile([P, 128], f16, tag="oh", name="oh")
                nc.vector.tensor_scalar(
                    out=oh[:], in0=iota_s[:],
                    scalar1=colf_s[:, s * SB * KREM + c:
                                   s * SB * KREM + c + 1],
                    scalar2=None, op0=Alu.is_equal)
            continue

        pag = psag.tile([P, SB * 128], f32, tag="ag", name="ag",
                        bufs=TUNE["agbufs"])
        pqs = []
        if TUNE["s1_first"]:
            for b4 in range(SB):
                ebase = (b4 * KTOT) * 128
                pq = pspq.tile([P, 512], f32, tag="pq", name="pq",
                               bufs=TUNE["pqbufs"])
                pqs.append(pq)
                for t in range(KMAIN):
                    i, q = t % 4, t // 4
                    nc.tensor.matmul(
                        out=pq[32 * i:32 * i + 32, 128 * q:128 * q + 128],
                        lhsT=R[:],
                        rhs=eblk[:, ebase + t * 128:ebase + (t + 1) * 128],
                        tile_position=(0, 32 * i), start=True, stop=True)
        for b4 in range(SB):
            ebase = (b4 * KTOT) * 128
            if TUNE["s1_first"]:
                pq = pqs[b4]
            else:
                # stage 1: 16 main tiles -> [128,512] groups (col-tiled 4x)
                pq = pspq.tile([P, 512], f32, tag="pq", name="pq",
                               bufs=TUNE["pqbufs"])
                for t in range(KMAIN):
                    i, q = t % 4, t // 4
                    nc.tensor.matmul(
                        out=pq[32 * i:32 * i + 32, 128 * q:128 * q + 128],
                        lhsT=R[:],
                        rhs=eblk[:, ebase + t * 128:ebase + (t + 1) * 128],
                        tile_position=(0, 32 * i), start=True, stop=True)
            if only == "s1":
                continue
            grp = spool.tile([P, 512], f16, tag="grp", name="grp")
            geng = nc.vector if b4 < TUNE["grp_dve"] else nc.scalar
            if geng is nc.vector:
                nc.vector.tensor_copy(grp[:], pq[:])
            else:
                nc.scalar.copy(grp[:], pq[:])
            # stage 2: 4 constant-scatter matmuls + 2 remainder edge tiles
            cbase = s * SB * KREM + b4 * KREM
            for q in range(SB):
                nc.tensor.matmul(out=pag[:, b4 * 128:(b4 + 1) * 128],
                                 lhsT=grp[:, q * 128:(q + 1) * 128],
                                 rhs=S[:, q, :], start=(q == 0), stop=False)
            for r in range(KREM):
                oh = ohpool.tile([P, 128], f16, tag="oh", name="oh")
                nc.vector.tensor_scalar(
                    out=oh[:], in0=iota_s[:],
                    scalar1=colf_s[:, cbase + r:cbase + r + 1],
                    scalar2=None, op0=Alu.is_equal)
                nc.tensor.matmul(
                    out=pag[:, b4 * 128:(b4 + 1) * 128],
                    lhsT=eblk[:, ebase + (KMAIN + r) * 128:
                              ebase + (KMAIN + r + 1) * 128],
                    rhs=oh[:], start=False, stop=(r == KREM - 1))
        if only in ("agg", "s1"):
            continue

        aggrT = spool.tile([P, SB * 128], f16, tag="aggrT", name="aggrT")
        nc.scalar.copy(aggrT[:], pag[:])

        # MLP (fp16 weights, fp32 PSUM accumulate)
        ph1 = psmlp.tile([P, SB * 128], f32, tag="mlp", name="mlp")
        nc.tensor.matmul(out=ph1[:], lhsT=consts["w0a"][:],
                         rhs=natT_s[:, s * SB * 128:(s + 1) * SB * 128],
                         start=True, stop=False)
        nc.tensor.matmul(out=ph1[:], lhsT=consts["w0b"][:], rhs=aggrT[:],
                         start=False, stop=True)
        h1 = spool.tile([P, SB * 128], f16, tag="h1", name="h1")
        nc.scalar.activation(h1[:], ph1[:], Act.Relu, bias=consts["b0"][:])

        ph2 = psmlp.tile([P, SB * 128], f32, tag="mlp", name="mlp")
        nc.tensor.matmul(out=ph2[:], lhsT=consts["w1"][:], rhs=h1[:],
                         start=True, stop=True)
        h2 = spool.tile([P, SB * 128], f16, tag="h2", name="h2")
        nc.scalar.activation(h2[:], ph2[:], Act.Relu, bias=consts["b1"][:])

        ph3 = psmlp.tile([P, SB * 128], f32, tag="mlp", name="mlp")
        nc.tensor.matmul(out=ph3[:], lhsT=consts["w2"][:], rhs=h2[:],
                         start=True, stop=True)
        h3T = spool.tile([P, SB * 128], f16, tag="h3T", name="h3T")
        nc.scalar.activation(h3T[:], ph3[:], Act.Identity,
                             bias=consts["b2"][:])
        sq = spool.tile([P, SB * 128], f16, tag="sq", name="sq")
        if TUNE["sq_engine"] == "dve":
            nc.vector.tensor_tensor(out=sq[:], in0=h3T[:], in1=h3T[:],
                                    op=Alu.mult)
        else:
            nc.scalar.activation(sq[:], h3T[:], Act.Square)

        # column stats: mu and E[x^2] per node into one aux PSUM bank
        paux = psaux.tile([P, 2 * SB], f32, tag="aux", name="aux")
        for b4 in range(SB):
            nc.tensor.matmul(out=paux[:, b4:b4 + 1],
                             lhsT=h3T[:, b4 * 128:(b4 + 1) * 128],
                             rhs=onesc[:], start=True, stop=True)
        for b4 in range(SB):
            nc.tensor.matmul(out=paux[:, SB + b4:SB + b4 + 1],
                             lhsT=sq[:, b4 * 128:(b4 + 1) * 128],
                             rhs=onesc[:], start=True, stop=True)
        mu_sb = spool.tile([P, SB], f32, tag="mu", name="mu")
        nc.scalar.copy(mu_sb[:], paux[:, 0:SB])
        musq = spool.tile([P, SB], f32, tag="musq", name="musq")
        nc.vector.tensor_tensor(out=musq[:], in0=mu_sb[:], in1=mu_sb[:],
                                op=Alu.mult)
        var = spool.tile([P, SB], f32, tag="var", name="var")
        nc.vector.tensor_tensor(out=var[:], in0=paux[:, SB:2 * SB],
                                in1=musq[:], op=Alu.subtract)
        std = spool.tile([P, SB], f32, tag="std", name="std")
        nc.scalar.activation(std[:], var[:], Act.Sqrt, bias=epst[:])
        rstd = spool.tile([P, SB], f32, tag="rstd", name="rstd")
        nc.vector.reciprocal(rstd[:], std[:])

        pyt = psag.tile([P, SB, 128], f32, tag="py", name="py", bufs=2)
        for b4 in range(SB):
            nc.tensor.matmul(out=pyt[:, b4, :],
                             lhsT=h3T[:, b4 * 128:(b4 + 1) * 128],
                             rhs=consts["ident"][:], start=True, stop=True)
        xn = spool.tile([P, SB, 128], f16, tag="xn", name="xn")
        for b4 in range(SB):
            nc.vector.tensor_scalar(
                out=xn[:, b4, :], in0=pyt[:, b4, :],
                scalar1=mu_sb[:, b4:b4 + 1], scalar2=rstd[:, b4:b4 + 1],
                op0=Alu.subtract, op1=Alu.mult)
        geng = nc.gpsimd if TUNE["gam_engine"] == "gp" else nc.vector
        beng = nc.gpsimd if TUNE["beta_engine"] == "gp" else nc.vector
        yg = spool.tile([P, SB, 128], f16, tag="yg", name="yg")
        geng.tensor_tensor(out=yg[:], in0=xn[:], in1=consts["gam"][:],
                           op=Alu.mult)
        yo = spool.tile([P, SB, 128], f16, tag="yo", name="yo")
        beng.tensor_tensor(out=yo[:], in0=yg[:], in1=consts["bet"][:],
                           op=Alu.add)
        odma.dma_start(out=out[s], in_=yo[:])


def _prepare_shards(node_attr, edge_attr, col):
    """Fixed 16-slot-per-node main region + LPT on overflow for remainder."""
    import heapq

    deg = np.bincount(col, minlength=NUM_NODES).astype(np.int64)
    over = np.maximum(deg - CAP, 0)
    order_nodes = np.argsort(-over, kind="stable")
    heap = [(0, 0, b) for b in range(TOTAL_BLOCKS)]
    heapq.heapify(heap)
    block_nodes = [[] for _ in range(TOTAL_BLOCKS)]
    for nd in order_nodes:
        d = int(over[nd])
        s, cnt, b = heapq.heappop(heap)
        block_nodes[b].append(int(nd))
        if cnt + 1 < BLK:
            heapq.heappush(heap, (s + d, cnt + 1, b))
    rem_max = max(sum(int(over[nd]) for nd in bn) for bn in block_nodes)
    assert rem_max <= KREM * 128, rem_max

    pos = np.full(NUM_NODES, -1, dtype=np.int64)      # old -> new node id
    natp = np.zeros((TOTAL_BLOCKS * BLK, D), np.float16)
    for b, bn in enumerate(block_nodes):
        ids = np.asarray(bn, dtype=np.int64)
        pos[ids] = b * BLK + np.arange(len(ids))
        natp[b * BLK:b * BLK + len(ids)] = node_attr[ids].astype(np.float16)
    assert (pos >= 0).all()

    # per-edge slot assignment
    order = np.argsort(col, kind="stable")           # edges grouped per node
    cs = col[order]
    within = np.arange(col.shape[0], dtype=np.int64)
    starts = np.zeros(NUM_NODES + 1, np.int64)
    starts[1:] = np.cumsum(deg)
    within = within - starts[cs]                     # rank within node
    npos = pos[cs]
    blk = npos >> 7
    loc = npos & 127

    main_mask = within < CAP
    slot = np.empty(col.shape[0], dtype=np.int64)
    slot[main_mask] = (blk[main_mask] * KTOT * 128 + loc[main_mask] * CAP +
                       within[main_mask])
    # overflow edges: sequential within their block's remainder region
    om = ~main_mask
    oblk = blk[om]
    oord = np.argsort(oblk, kind="stable")
    ocnt = np.bincount(oblk, minlength=TOTAL_BLOCKS)
    ostart = np.zeros(TOTAL_BLOCKS + 1, np.int64)
    ostart[1:] = np.cumsum(ocnt)
    opos_in_blk = np.arange(om.sum(), dtype=np.int64) - ostart[oblk[oord]]
    oslot = np.empty(om.sum(), dtype=np.int64)
    oslot[oord] = (oblk[oord] * KTOT * 128 + KMAIN * 128 + opos_in_blk)
    slot[om] = oslot

    ea16 = edge_attr.astype(np.float16)
    slots_per_core = BLOCKS_PER_CORE * KTOT * 128
    edges_by_core = []
    colf_by_core = []
    natT_by_core = []
    blk_of = slot // (KTOT * 128)
    off_of = slot % (KTOT * 128)
    loc_f = loc.astype(np.float32)
    for c in range(N_CORES):
        sel = (blk_of >= c * BLOCKS_PER_CORE) & \
              (blk_of < (c + 1) * BLOCKS_PER_CORE)
        lblk = blk_of[sel] - c * BLOCKS_PER_CORE
        lslot = lblk * (KTOT * 128) + off_of[sel]
        ebuf = np.zeros((slots_per_core, D), np.float16)
        ebuf[lslot] = ea16[order[sel]]
        earr = np.ascontiguousarray(
            ebuf.reshape(BLOCKS_PER_CORE * KTOT, 128, D)
            .transpose(1, 0, 2).reshape(P, slots_per_core))
        edges_by_core.append(earr)
        cf = np.full((BLOCKS_PER_CORE, KREM, 128), -1.0, np.float32)
        rm = off_of[sel] >= KMAIN * 128
        roff = off_of[sel][rm] - KMAIN * 128
        cf[lblk[rm], roff // 128, roff % 128] = loc_f[sel][rm]
        carr = np.ascontiguousarray(
            cf.reshape(BLOCKS_PER_CORE * KREM, 128).T)
        colf_by_core.append(carr)
        natT_by_core.append(np.ascontiguousarray(
            natp[c * NODES_PER_CORE:(c + 1) * NODES_PER_CORE].T))
    kb = (KTOT,) * BLOCKS_PER_CORE
    return kb, edges_by_core, colf_by_core, natT_by_core, pos


def kernel(node_attr, edge_attr, edge_index, W0, b0, W1, b1, W2, b2,
           ln_g, ln_b):
    from concourse import bass_utils

    node_attr = np.ascontiguousarray(np.asarray(node_attr, dtype=np.float32))
    edge_attr = np.ascontiguousarray(np.asarray(edge_attr, dtype=np.float32))
    col = np.asarray(edge_index)[1].astype(np.int64)
    W0 = np.asarray(W0, dtype=np.float32)
    W1 = np.asarray(W1, dtype=np.float32)
    W2 = np.asarray(W2, dtype=np.float32)
    b0v = np.asarray(b0, np.float32).reshape(128, 1).copy()
    b1v = np.asarray(b1, np.float32).reshape(128, 1).copy()
    b2v = np.asarray(b2, np.float32).reshape(128, 1).copy()
    gam = np.ascontiguousarray(
        np.broadcast_to(np.asarray(ln_g, np.float32).reshape(1, 1, 128),
                        (128, SB, 128)).astype(np.float16))
    bet = np.ascontiguousarray(
        np.broadcast_to(np.asarray(ln_b, np.float32).reshape(1, 1, 128),
                        (128, SB, 128)).astype(np.float16))

    kb, edges_by_core, colf_by_core, natT_by_core, pos = _prepare_shards(
        node_attr, edge_attr, col)

    iota_rep = np.ascontiguousarray(
        np.broadcast_to(np.arange(128, dtype=np.float16), (P, 128)))
    rmat = np.zeros((128, 32), np.float16)
    rmat[np.arange(128), np.arange(128) // G] = 1.0
    # S_q[p, j] = 1 iff node j owns group at partition p of quad-column q:
    #   j = 32 q + 8 (p//32) + (p%32)//4
    smat = np.zeros((128, SB, 128), np.float16)
    pidx = np.arange(128)
    for q in range(SB):
        smat[pidx, q, 32 * q + 8 * (pidx // 32) + (pidx % 32) // 4] = 1.0
    ident = np.eye(128, dtype=np.float16)

    if kb not in _nc_cache:
        _nc_cache[kb] = _build_nc(kb)
    nc = _nc_cache[kb]

    shared = {"iota": iota_rep, "rmat": rmat, "smat": smat, "ident": ident,
              "w0a": np.ascontiguousarray(W0[:128].astype(np.float16)),
              "w0b": np.ascontiguousarray(W0[128:].astype(np.float16)),
              "w1": np.ascontiguousarray(W1.astype(np.float16)),
              "w2": np.ascontiguousarray(W2.astype(np.float16)),
              "gam": gam, "bet": bet, "b0": b0v, "b1": b1v, "b2": b2v}
    in_maps = []
    for c in range(N_CORES):
        m = {"edges": edges_by_core[c], "colf32": colf_by_core[c],
             "natT": natT_by_core[c]}
        m.update(shared)
        in_maps.append(m)

    res = bass_utils.run_bass_kernel_spmd(nc, in_maps,
                                          core_ids=list(range(N_CORES)))
    last_run_info["results"] = res
    last_run_info["nc"] = nc
    last_run_info["in_maps"] = in_maps
    last_run_info["kb"] = kb
    last_run_info["pos"] = pos

    rows = np.concatenate(
        [res.results[c]["out"].reshape(SBLOCKS, P, SB, 128)
         .transpose(0, 2, 1, 3).reshape(NODES_PER_CORE, D)
         for c in range(N_CORES)], axis=0)
    return rows[pos].astype(np.float32)


if __name__ == "__main__":
    pass


# revision 18
# speedup vs baseline: 1.3663x; 1.3482x over previous
"""Trainium2 Bass kernel for nn_NodeBlock (GNN message passing), v4.

Pipeline: segment_sum of edge features onto destination nodes, concat with
node features, 3-layer MLP, LayerNorm.

Layout: nodes are packed into 800 blocks of 128 (LPT on overflow degree),
blocks dealt to 8 cores.  Every node owns a FIXED span of 16 "main" edge
slots (4 groups of G=4); edges beyond 16 per node go to a per-block
remainder region (up to 256 slots).  Edges are fp16 (~7e-4 rel err vs the
2e-2 gate).

Segment sum is two-stage with constant matrices: per block, 16 main edge
tiles are pre-reduced on the PE with a fixed G=4 grouping matrix R
(col-tiled 4x: 32-col matmuls at tile_position (0,32i) -> one [128,512]
PSUM of per-group sums); because group->node is fixed, the scatter of the
512 groups onto 128 nodes is 4 matmuls against a CONSTANT selection matrix
S_q (no per-tile DVE work).  Only the <=256 remainder edges per block use
DVE one-hot (is_equal vs iota) scatter.

Four blocks form a superblock (512 nodes) with a fused fp16 MLP + LayerNorm
epilogue: column stats via stationary matmuls against a 1/128 ones column,
PE transposes to node-major, normalize via DVE tensor_scalar, gamma/beta
TTs.  Output is fp16, node-permuted; the host undoes the permutation.
"""

import sys

sys.path.insert(0, "/opt/trn_rl_repo")

import numpy as np

N_CORES = 8
NUM_NODES = 100000
D = 128            # node/edge feature dim
P = 128            # partitions
BLK = 128          # nodes per block
G = 4              # edges per pre-reduction group
CAP = 16           # main edge slots per node (4 groups)
KMAIN = 16         # main edge tiles per block (= BLK*CAP/128, 4 quads)
KREM = 2           # remainder edge tiles per block (direct one-hot)
KTOT = KMAIN + KREM
BLOCKS_PER_CORE = 100
SB = 4             # blocks per superblock
SBLOCKS = BLOCKS_PER_CORE // SB          # 25
NODES_PER_CORE = BLK * BLOCKS_PER_CORE   # 12800
TOTAL_BLOCKS = N_CORES * BLOCKS_PER_CORE  # 800
EPS = 1e-5

_nc_cache = {}
last_run_info = {}

TUNE = {"ebufs": 3, "ohbufs": 8, "sbufs": 3, "agbufs": 1, "mlpbufs": 1,
        "pqbufs": 4, "gam_engine": "dve", "beta_engine": "dve",
        "sq_engine": "dve", "grp_dve": 2, "s1_first": True, "rem_first": False,
        "only": None}


def _build_nc(kb, loop_iters=None):
    """kb is kept for test.py compatibility; v4 uses fixed KTOT tiles."""
    import contextlib
    import concourse.bacc as bacc
    import concourse.tile as tile
    import concourse.mybir as mybir

    dt = mybir.dt
    f32 = dt.float32
    f16 = dt.float16
    tot_e = BLOCKS_PER_CORE * KTOT * 128

    nc = bacc.Bacc("TRN2", target_bir_lowering=False, debug=False,
                   name="nodeblock")

    edges = nc.dram_tensor("edges", [P, tot_e], f16, kind="ExternalInput")
    colf32 = nc.dram_tensor("colf32", [P, BLOCKS_PER_CORE * KREM],
                            f32, kind="ExternalInput")
    natT = nc.dram_tensor("natT", [P, NODES_PER_CORE], f16,
                          kind="ExternalInput")
    iota = nc.dram_tensor("iota", [P, 128], f16, kind="ExternalInput")
    w_in = {}
    for nm in ["w0a", "w0b", "w1", "w2", "ident"]:
        w_in[nm] = nc.dram_tensor(nm, [128, 128], f16, kind="ExternalInput")
    for nm in ["b0", "b1", "b2"]:
        w_in[nm] = nc.dram_tensor(nm, [128, 1], f32, kind="ExternalInput")
    w_in["rmat"] = nc.dram_tensor("rmat", [128, 32], f16,
                                  kind="ExternalInput")
    w_in["smat"] = nc.dram_tensor("smat", [128, SB, 128], f16,
                                  kind="ExternalInput")
    w_in["gam"] = nc.dram_tensor("gam", [128, SB, 128], f16,
                                 kind="ExternalInput")
    w_in["bet"] = nc.dram_tensor("bet", [128, SB, 128], f16,
                                 kind="ExternalInput")
    out = nc.dram_tensor("out", [SBLOCKS, P, SB, 128], f16,
                         kind="ExternalOutput")

    with tile.TileContext(nc) as tc:
        with (
            tc.tile_pool(name="const", bufs=1) as cpool,
            tc.tile_pool(name="edge", bufs=TUNE["ebufs"]) as epool,
            tc.tile_pool(name="oh", bufs=TUNE["ohbufs"]) as ohpool,
            tc.tile_pool(name="small", bufs=TUNE["sbufs"]) as spool,
            tc.tile_pool(name="psag", bufs=TUNE["agbufs"],
                         space="PSUM") as psag,
            tc.tile_pool(name="psmlp", bufs=TUNE["mlpbufs"],
                         space="PSUM") as psmlp,
            tc.tile_pool(name="pspq", bufs=TUNE["pqbufs"],
                         space="PSUM") as pspq,
            tc.tile_pool(name="psaux", bufs=1, space="PSUM") as psaux,
        ):
            cdma = nc.scalar
            colf_s = cpool.tile([P, BLOCKS_PER_CORE * KREM], f32,
                                tag="colf32", name="colf32")
            cdma.dma_start(out=colf_s[:], in_=colf32[:])
            natT_s = cpool.tile([P, NODES_PER_CORE], f16, tag="natT",
                                name="natT")
            cdma.dma_start(out=natT_s[:], in_=natT[:])
            iota_s = cpool.tile([P, 128], f16, tag="iota", name="iota")
            cdma.dma_start(out=iota_s[:], in_=iota[:])
            consts = {}
            for nm, t in w_in.items():
                dtt = f32 if nm in ("b0", "b1", "b2") else f16
                consts[nm] = cpool.tile(list(t.shape), dtt, tag=nm, name=nm)
                cdma.dma_start(out=consts[nm][:], in_=t[:])
            onesc = cpool.tile([P, 1], f16, tag="onesc", name="onesc")
            nc.vector.memset(onesc[:], 1.0 / 128.0)
            epst = cpool.tile([P, 1], f32, tag="eps", name="eps")
            nc.vector.memset(epst[:], EPS)

            loop_cm = (tc.For_i(0, loop_iters, 1) if loop_iters
                       else contextlib.nullcontext())
            with loop_cm:
                _emit(nc, tc, epool, ohpool, spool, psag, psmlp, pspq,
                      psaux, colf_s, natT_s, iota_s, consts, onesc, epst,
                      edges, out, mybir)
    nc.finalize()
    return nc


def _emit(nc, tc, epool, ohpool, spool, psag, psmlp, pspq, psaux, colf_s,
          natT_s, iota_s, consts, onesc, epst, edges, out, mybir):
    dt = mybir.dt
    f32 = dt.float32
    f16 = dt.float16
    Alu = mybir.AluOpType
    Act = mybir.ActivationFunctionType
    only = TUNE["only"]
    edma = nc.sync
    odma = nc.scalar
    R = consts["rmat"]
    S = consts["smat"]
    sb_e = SB * KTOT * 128          # edge elems per superblock per partition

    for s in range(SBLOCKS):
        if only in (None, "dma", "agg", "s1"):
            eblk = epool.tile([P, sb_e], f16, tag="eblk", name="eblk")
            edma.dma_start(out=eblk[:], in_=edges[:, s * sb_e:(s + 1) * sb_e])
        if only == "dma":
            continue

        if only == "dve":
            for c in range(SB * KREM):
                oh = ohpool.tile([P, 128], f16, tag="oh", name="oh")
                nc.vector.tensor_scalar(
                    out=oh[:], in0=iota_s[:],
                    scalar1=colf_s[:, s * SB * KREM + c:
                                   s * SB * KREM + c + 1],
                    scalar2=None, op0=Alu.is_equal)
            continue

        pag = psag.tile([P, SB * 128], f32, tag="ag", name="ag",
                        bufs=TUNE["agbufs"])
        pqs = []
        if TUNE["s1_first"]:
            for b4 in range(SB):
                ebase = (b4 * KTOT) * 128
                pq = pspq.tile([P, 512], f32, tag="pq", name="pq",
                               bufs=TUNE["pqbufs"])
                pqs.append(pq)
                for t in range(KMAIN):
                    i, q = t % 4, t // 4
                    nc.tensor.matmul(
                        out=pq[32 * i:32 * i + 32, 128 * q:128 * q + 128],
                        lhsT=R[:],
                        rhs=eblk[:, ebase + t * 128:ebase + (t + 1) * 128],
                        tile_position=(0, 32 * i), start=True, stop=True)
        for b4 in range(SB):
            ebase = (b4 * KTOT) * 128
            if TUNE["s1_first"]:
                pq = pqs[b4]
            else:
                # stage 1: 16 main tiles -> [128,512] groups (col-tiled 4x)
                pq = pspq.tile([P, 512], f32, tag="pq", name="pq",
                               bufs=TUNE["pqbufs"])
                for t in range(KMAIN):
                    i, q = t % 4, t // 4
                    nc.tensor.matmul(
                        out=pq[32 * i:32 * i + 32, 128 * q:128 * q + 128],
                        lhsT=R[:],
                        rhs=eblk[:, ebase + t * 128:ebase + (t + 1) * 128],
                        tile_position=(0, 32 * i), start=True, stop=True)
            if only == "s1":
                continue
            grp = spool.tile([P, 512], f16, tag="grp", name="grp")
            geng = nc.vector if b4 < TUNE["grp_dve"] else nc.scalar
            if geng is nc.vector:
                nc.vector.tensor_copy(grp[:], pq[:])
            else:
                nc.scalar.copy(grp[:], pq[:])
            # stage 2: 2 remainder one-hot matmuls open the accumulation,
            # then 4 constant-scatter matmuls close it (no DVE deps late).
            cbase = s * SB * KREM + b4 * KREM
            if TUNE["rem_first"]:
                for r in range(KREM):
                    oh = ohpool.tile([P, 128], f16, tag="oh", name="oh")
                    nc.vector.tensor_scalar(
                        out=oh[:], in0=iota_s[:],
                        scalar1=colf_s[:, cbase + r:cbase + r + 1],
                        scalar2=None, op0=Alu.is_equal)
                    nc.tensor.matmul(
                        out=pag[:, b4 * 128:(b4 + 1) * 128],
                        lhsT=eblk[:, ebase + (KMAIN + r) * 128:
                                  ebase + (KMAIN + r + 1) * 128],
                        rhs=oh[:], start=(r == 0), stop=False)
                for q in range(SB):
                    nc.tensor.matmul(out=pag[:, b4 * 128:(b4 + 1) * 128],
                                     lhsT=grp[:, q * 128:(q + 1) * 128],
                                     rhs=S[:, q, :], start=False,
                                     stop=(q == SB - 1))
            else:
                for q in range(SB):
                    nc.tensor.matmul(out=pag[:, b4 * 128:(b4 + 1) * 128],
                                     lhsT=grp[:, q * 128:(q + 1) * 128],
                                     rhs=S[:, q, :], start=(q == 0),
                                     stop=False)
                for r in range(KREM):
                    oh = ohpool.tile([P, 128], f16, tag="oh", name="oh")
                    nc.vector.tensor_scalar(
                        out=oh[:], in0=iota_s[:],
                        scalar1=colf_s[:, cbase + r:cbase + r + 1],
                        scalar2=None, op0=Alu.is_equal)
                    nc.tensor.matmul(
                        out=pag[:, b4 * 128:(b4 + 1) * 128],
                        lhsT=eblk[:, ebase + (KMAIN + r) * 128:
                                  ebase + (KMAIN + r + 1) * 128],
                        rhs=oh[:], start=False, stop=(r == KREM - 1))
        if only in ("agg", "s1"):
            continue

        aggrT = spool.tile([P, SB * 128], f16, tag="aggrT", name="aggrT")
        nc.scalar.copy(aggrT[:], pag[:])

        # MLP (fp16 weights, fp32 PSUM accumulate)
        ph1 = psmlp.tile([P, SB * 128], f32, tag="mlp", name="mlp")
        nc.tensor.matmul(out=ph1[:], lhsT=consts["w0a"][:],
                         rhs=natT_s[:, s * SB * 128:(s + 1) * SB * 128],
                         start=True, stop=False)
        nc.tensor.matmul(out=ph1[:], lhsT=consts["w0b"][:], rhs=aggrT[:],
                         start=False, stop=True)
        h1 = spool.tile([P, SB * 128], f16, tag="h1", name="h1")
        nc.scalar.activation(h1[:], ph1[:], Act.Relu, bias=consts["b0"][:])

        ph2 = psmlp.tile([P, SB * 128], f32, tag="mlp", name="mlp")
        nc.tensor.matmul(out=ph2[:], lhsT=consts["w1"][:], rhs=h1[:],
                         start=True, stop=True)
        h2 = spool.tile([P, SB * 128], f16, tag="h2", name="h2")
        nc.scalar.activation(h2[:], ph2[:], Act.Relu, bias=consts["b1"][:])

        ph3 = psmlp.tile([P, SB * 128], f32, tag="mlp", name="mlp")
        nc.tensor.matmul(out=ph3[:], lhsT=consts["w2"][:], rhs=h2[:],
                         start=True, stop=True)
        h3T = spool.tile([P, SB * 128], f16, tag="h3T", name="h3T")
        nc.scalar.activation(h3T[:], ph3[:], Act.Identity,
                             bias=consts["b2"][:])

        pyt = psag.tile([P, SB, 128], f32, tag="py", name="py", bufs=2)
        for b4 in range(SB):
            nc.tensor.matmul(out=pyt[:, b4, :],
                             lhsT=h3T[:, b4 * 128:(b4 + 1) * 128],
                             rhs=consts["ident"][:], start=True, stop=True)
        # node-major LayerNorm stats straight off the PSUM transpose
        stats = spool.tile([P, SB, 6], f32, tag="stats", name="stats")
        for b4 in range(SB):
            nc.vector.bn_stats(stats[:, b4, :], pyt[:, b4, :])
        mv = spool.tile([P, SB, 2], f32, tag="mv", name="mv")
        for b4 in range(SB):
            nc.vector.bn_aggr(mv[:, b4, :], stats[:, b4, :])
        std = spool.tile([P, SB], f32, tag="std", name="std")
        nc.scalar.activation(std[:], mv[:, :, 1], Act.Sqrt, bias=epst[:])
        rstd = spool.tile([P, SB], f32, tag="rstd", name="rstd")
        nc.vector.reciprocal(rstd[:], std[:])
        xn = spool.tile([P, SB, 128], f16, tag="xn", name="xn")
        for b4 in range(SB):
            nc.vector.tensor_scalar(
                out=xn[:, b4, :], in0=pyt[:, b4, :],
                scalar1=mv[:, b4, 0:1], scalar2=rstd[:, b4:b4 + 1],
                op0=Alu.subtract, op1=Alu.mult)
        geng = nc.gpsimd if TUNE["gam_engine"] == "gp" else nc.vector
        beng = nc.gpsimd if TUNE["beta_engine"] == "gp" else nc.vector
        yg = spool.tile([P, SB, 128], f16, tag="yg", name="yg")
        geng.tensor_tensor(out=yg[:], in0=xn[:], in1=consts["gam"][:],
                           op=Alu.mult)
        yo = spool.tile([P, SB, 128], f16, tag="yo", name="yo")
        beng.tensor_tensor(out=yo[:], in0=yg[:], in1=consts["bet"][:],
                           op=Alu.add)
        odma.dma_start(out=out[s], in_=yo[:])


def _prepare_shards(node_attr, edge_attr, col):
    """Fixed 16-slot-per-node main region + LPT on overflow for remainder."""
    import heapq

    deg = np.bincount(col, minlength=NUM_NODES).astype(np.int64)
    over = np.maximum(deg - CAP, 0)
    order_nodes = np.argsort(-over, kind="stable")
    heap = [(0, 0, b) for b in range(TOTAL_BLOCKS)]
    heapq.heapify(heap)
    block_nodes = [[] for _ in range(TOTAL_BLOCKS)]
    for nd in order_nodes:
        d = int(over[nd])
        s, cnt, b = heapq.heappop(heap)
        block_nodes[b].append(int(nd))
        if cnt + 1 < BLK:
            heapq.heappush(heap, (s + d, cnt + 1, b))
    rem_max = max(sum(int(over[nd]) for nd in bn) for bn in block_nodes)
    assert rem_max <= KREM * 128, rem_max

    pos = np.full(NUM_NODES, -1, dtype=np.int64)      # old -> new node id
    natp = np.zeros((TOTAL_BLOCKS * BLK, D), np.float16)
    for b, bn in enumerate(block_nodes):
        ids = np.asarray(bn, dtype=np.int64)
        pos[ids] = b * BLK + np.arange(len(ids))
        natp[b * BLK:b * BLK + len(ids)] = node_attr[ids].astype(np.float16)
    assert (pos >= 0).all()

    # per-edge slot assignment
    order = np.argsort(col, kind="stable")           # edges grouped per node
    cs = col[order]
    within = np.arange(col.shape[0], dtype=np.int64)
    starts = np.zeros(NUM_NODES + 1, np.int64)
    starts[1:] = np.cumsum(deg)
    within = within - starts[cs]                     # rank within node
    npos = pos[cs]
    blk = npos >> 7
    loc = npos & 127

    main_mask = within < CAP
    slot = np.empty(col.shape[0], dtype=np.int64)
    slot[main_mask] = (blk[main_mask] * KTOT * 128 + loc[main_mask] * CAP +
                       within[main_mask])
    # overflow edges: sequential within their block's remainder region
    om = ~main_mask
    oblk = blk[om]
    oord = np.argsort(oblk, kind="stable")
    ocnt = np.bincount(oblk, minlength=TOTAL_BLOCKS)
    ostart = np.zeros(TOTAL_BLOCKS + 1, np.int64)
    ostart[1:] = np.cumsum(ocnt)
    opos_in_blk = np.arange(om.sum(), dtype=np.int64) - ostart[oblk[oord]]
    oslot = np.empty(om.sum(), dtype=np.int64)
    oslot[oord] = (oblk[oord] * KTOT * 128 + KMAIN * 128 + opos_in_blk)
    slot[om] = oslot

    ea16 = edge_attr.astype(np.float16)
    slots_per_core = BLOCKS_PER_CORE * KTOT * 128
    edges_by_core = []
    colf_by_core = []
    natT_by_core = []
    blk_of = slot // (KTOT * 128)
    off_of = slot % (KTOT * 128)
    loc_f = loc.astype(np.float32)
    for c in range(N_CORES):
        sel = (blk_of >= c * BLOCKS_PER_CORE) & \
              (blk_of < (c + 1) * BLOCKS_PER_CORE)
        lblk = blk_of[sel] - c * BLOCKS_PER_CORE
        lslot = lblk * (KTOT * 128) + off_of[sel]
        ebuf = np.zeros((slots_per_core, D), np.float16)
        ebuf[lslot] = ea16[order[sel]]
        earr = np.ascontiguousarray(
            ebuf.reshape(BLOCKS_PER_CORE * KTOT, 128, D)
            .transpose(1, 0, 2).reshape(P, slots_per_core))
        edges_by_core.append(earr)
        cf = np.full((BLOCKS_PER_CORE, KREM, 128), -1.0, np.float32)
        rm = off_of[sel] >= KMAIN * 128
        roff = off_of[sel][rm] - KMAIN * 128
        cf[lblk[rm], roff // 128, roff % 128] = loc_f[sel][rm]
        carr = np.ascontiguousarray(
            cf.reshape(BLOCKS_PER_CORE * KREM, 128).T)
        colf_by_core.append(carr)
        natT_by_core.append(np.ascontiguousarray(
            natp[c * NODES_PER_CORE:(c + 1) * NODES_PER_CORE].T))
    kb = (KTOT,) * BLOCKS_PER_CORE
    return kb, edges_by_core, colf_by_core, natT_by_core, pos


def kernel(node_attr, edge_attr, edge_index, W0, b0, W1, b1, W2, b2,
           ln_g, ln_b):
    from concourse import bass_utils

    node_attr = np.ascontiguousarray(np.asarray(node_attr, dtype=np.float32))
    edge_attr = np.ascontiguousarray(np.asarray(edge_attr, dtype=np.float32))
    col = np.asarray(edge_index)[1].astype(np.int64)
    W0 = np.asarray(W0, dtype=np.float32)
    W1 = np.asarray(W1, dtype=np.float32)
    W2 = np.asarray(W2, dtype=np.float32)
    b0v = np.asarray(b0, np.float32).reshape(128, 1).copy()
    b1v = np.asarray(b1, np.float32).reshape(128, 1).copy()
    b2v = np.asarray(b2, np.float32).reshape(128, 1).copy()
    gam = np.ascontiguousarray(
        np.broadcast_to(np.asarray(ln_g, np.float32).reshape(1, 1, 128),
                        (128, SB, 128)).astype(np.float16))
    bet = np.ascontiguousarray(
        np.broadcast_to(np.asarray(ln_b, np.float32).reshape(1, 1, 128),
                        (128, SB, 128)).astype(np.float16))

    kb, edges_by_core, colf_by_core, natT_by_core, pos = _prepare_shards(
        node_attr, edge_attr, col)

    iota_rep = np.ascontiguousarray(
        np.broadcast_to(np.arange(128, dtype=np.float16), (P, 128)))
    rmat = np.zeros((128, 32), np.float16)
    rmat[np.arange(128), np.arange(128) // G] = 1.0
    # S_q[p, j] = 1 iff node j owns group at partition p of quad-column q:
    #   j = 32 q + 8 (p//32) + (p%32)//4
    smat = np.zeros((128, SB, 128), np.float16)
    pidx = np.arange(128)
    for q in range(SB):
        smat[pidx, q, 32 * q + 8 * (pidx // 32) + (pidx % 32) // 4] = 1.0
    ident = np.eye(128, dtype=np.float16)

    if kb not in _nc_cache:
        _nc_cache[kb] = _build_nc(kb)
    nc = _nc_cache[kb]

    shared = {"iota": iota_rep, "rmat": rmat, "smat": smat, "ident": ident,
              "w0a": np.ascontiguousarray(W0[:128].astype(np.float16)),
              "w0b": np.ascontiguousarray(W0[128:].astype(np.float16)),
              "w1": np.ascontiguousarray(W1.astype(np.float16)),
              "w2": np.ascontiguousarray(W2.astype(np.float16)),
              "gam": gam, "bet": bet, "b0": b0v, "b1": b1v, "b2": b2v}
    in_maps = []
    for c in range(N_CORES):
        m = {"edges": edges_by_core[c], "colf32": colf_by_core[c],
             "natT": natT_by_core[c]}
        m.update(shared)
        in_maps.append(m)

    res = bass_utils.run_bass_kernel_spmd(nc, in_maps,
                                          core_ids=list(range(N_CORES)))
    last_run_info["results"] = res
    last_run_info["nc"] = nc
    last_run_info["in_maps"] = in_maps
    last_run_info["kb"] = kb
    last_run_info["pos"] = pos

    rows = np.concatenate(
        [res.results[c]["out"].reshape(SBLOCKS, P, SB, 128)
         .transpose(0, 2, 1, 3).reshape(NODES_PER_CORE, D)
         for c in range(N_CORES)], axis=0)
    return rows[pos].astype(np.float32)


if __name__ == "__main__":
    pass


# revision 21
# speedup vs baseline: 1.3685x; 1.0017x over previous
"""Trainium2 Bass kernel for nn_NodeBlock (GNN message passing), v4.

Pipeline: segment_sum of edge features onto destination nodes, concat with
node features, 3-layer MLP, LayerNorm.

Layout: nodes are packed into 800 blocks of 128 (LPT on overflow degree),
blocks dealt to 8 cores.  Every node owns a FIXED span of 16 "main" edge
slots (4 groups of G=4); edges beyond 16 per node go to a per-block
remainder region (up to 256 slots).  Edges are fp16 (~7e-4 rel err vs the
2e-2 gate).

Segment sum is two-stage with constant matrices: per block, 16 main edge
tiles are pre-reduced on the PE with a fixed G=4 grouping matrix R
(col-tiled 4x: 32-col matmuls at tile_position (0,32i) -> one [128,512]
PSUM of per-group sums); because group->node is fixed, the scatter of the
512 groups onto 128 nodes is 4 matmuls against a CONSTANT selection matrix
S_q (no per-tile DVE work).  Only the <=256 remainder edges per block use
DVE one-hot (is_equal vs iota) scatter.

Four blocks form a superblock (512 nodes) with a fused fp16 MLP + LayerNorm
epilogue: column stats via stationary matmuls against a 1/128 ones column,
PE transposes to node-major, normalize via DVE tensor_scalar, gamma/beta
TTs.  Output is fp16, node-permuted; the host undoes the permutation.
"""

import sys

sys.path.insert(0, "/opt/trn_rl_repo")

import numpy as np

N_CORES = 8
NUM_NODES = 100000
D = 128            # node/edge feature dim
P = 128            # partitions
BLK = 128          # nodes per block
G = 4              # edges per pre-reduction group
CAP = 16           # main edge slots per node (4 groups)
KMAIN = 16         # main edge tiles per block (= BLK*CAP/128, 4 quads)
KREM = 2           # remainder edge tiles per block (direct one-hot)
KTOT = KMAIN + KREM
BLOCKS_PER_CORE = 100
SB = 4             # blocks per superblock
SBLOCKS = BLOCKS_PER_CORE // SB          # 25
NODES_PER_CORE = BLK * BLOCKS_PER_CORE   # 12800
TOTAL_BLOCKS = N_CORES * BLOCKS_PER_CORE  # 800
EPS = 1e-5

_nc_cache = {}
last_run_info = {}

TUNE = {"ebufs": 3, "ohbufs": 8, "sbufs": 3, "agbufs": 1, "mlpbufs": 1,
        "pqbufs": 4, "gam_engine": "dve", "beta_engine": "dve",
        "sq_engine": "dve", "grp_dve": 2, "s1_first": True, "rem_first": False,
        "edge_pair": False, "grpbufs": 3, "aggrT_dve": 0,
        "only": None}


def _build_nc(kb, loop_iters=None):
    """kb is kept for test.py compatibility; v4 uses fixed KTOT tiles."""
    import contextlib
    import concourse.bacc as bacc
    import concourse.tile as tile
    import concourse.mybir as mybir

    dt = mybir.dt
    f32 = dt.float32
    f16 = dt.float16
    tot_e = BLOCKS_PER_CORE * KTOT * 128

    nc = bacc.Bacc("TRN2", target_bir_lowering=False, debug=False,
                   name="nodeblock")

    edges = nc.dram_tensor("edges", [P, tot_e], f16, kind="ExternalInput")
    colf32 = nc.dram_tensor("colf32", [P, BLOCKS_PER_CORE * KREM],
                            f32, kind="ExternalInput")
    natT = nc.dram_tensor("natT", [P, NODES_PER_CORE], f16,
                          kind="ExternalInput")
    iota = nc.dram_tensor("iota", [P, 128], f16, kind="ExternalInput")
    w_in = {}
    for nm in ["w0a", "w0b", "w1", "w2", "ident"]:
        w_in[nm] = nc.dram_tensor(nm, [128, 128], f16, kind="ExternalInput")
    for nm in ["b0", "b1", "b2"]:
        w_in[nm] = nc.dram_tensor(nm, [128, 1], f32, kind="ExternalInput")
    w_in["rmat"] = nc.dram_tensor("rmat", [128, 32], f16,
                                  kind="ExternalInput")
    w_in["smat"] = nc.dram_tensor("smat", [128, SB, 128], f16,
                                  kind="ExternalInput")
    w_in["gam"] = nc.dram_tensor("gam", [128, SB, 128], f16,
                                 kind="ExternalInput")
    w_in["bet"] = nc.dram_tensor("bet", [128, SB, 128], f16,
                                 kind="ExternalInput")
    out = nc.dram_tensor("out", [SBLOCKS, P, SB, 128], f16,
                         kind="ExternalOutput")

    with tile.TileContext(nc) as tc:
        with (
            tc.tile_pool(name="const", bufs=1) as cpool,
            tc.tile_pool(name="edge", bufs=TUNE["ebufs"]) as epool,
            tc.tile_pool(name="oh", bufs=TUNE["ohbufs"]) as ohpool,
            tc.tile_pool(name="small", bufs=TUNE["sbufs"]) as spool,
            tc.tile_pool(name="psag", bufs=TUNE["agbufs"],
                         space="PSUM") as psag,
            tc.tile_pool(name="psmlp", bufs=TUNE["mlpbufs"],
                         space="PSUM") as psmlp,
            tc.tile_pool(name="pspq", bufs=TUNE["pqbufs"],
                         space="PSUM") as pspq,
            tc.tile_pool(name="psaux", bufs=1, space="PSUM") as psaux,
        ):
            cdma = nc.scalar
            colf_s = cpool.tile([P, BLOCKS_PER_CORE * KREM], f32,
                                tag="colf32", name="colf32")
            cdma.dma_start(out=colf_s[:], in_=colf32[:])
            natT_s = cpool.tile([P, NODES_PER_CORE], f16, tag="natT",
                                name="natT")
            cdma.dma_start(out=natT_s[:], in_=natT[:])
            iota_s = cpool.tile([P, 128], f16, tag="iota", name="iota")
            cdma.dma_start(out=iota_s[:], in_=iota[:])
            consts = {}
            for nm, t in w_in.items():
                dtt = f32 if nm in ("b0", "b1", "b2") else f16
                consts[nm] = cpool.tile(list(t.shape), dtt, tag=nm, name=nm)
                cdma.dma_start(out=consts[nm][:], in_=t[:])
            onesc = cpool.tile([P, 1], f16, tag="onesc", name="onesc")
            nc.vector.memset(onesc[:], 1.0 / 128.0)
            epst = cpool.tile([P, 1], f32, tag="eps", name="eps")
            nc.vector.memset(epst[:], EPS)

            loop_cm = (tc.For_i(0, loop_iters, 1) if loop_iters
                       else contextlib.nullcontext())
            with loop_cm:
                _emit(nc, tc, epool, ohpool, spool, psag, psmlp, pspq,
                      psaux, colf_s, natT_s, iota_s, consts, onesc, epst,
                      edges, out, mybir)
    nc.finalize()
    return nc


def _emit(nc, tc, epool, ohpool, spool, psag, psmlp, pspq, psaux, colf_s,
          natT_s, iota_s, consts, onesc, epst, edges, out, mybir):
    dt = mybir.dt
    f32 = dt.float32
    f16 = dt.float16
    Alu = mybir.AluOpType
    Act = mybir.ActivationFunctionType
    only = TUNE["only"]
    edma = nc.sync
    odma = nc.scalar
    R = consts["rmat"]
    S = consts["smat"]
    sb_e = SB * KTOT * 128          # edge elems per superblock per partition

    pair_tile = None
    for s in range(SBLOCKS):
        if only in (None, "dma", "agg", "s1"):
            if TUNE["edge_pair"]:
                if s % 2 == 0:
                    span = min(2, SBLOCKS - s) * sb_e
                    pair_tile = epool.tile([P, 2 * sb_e], f16, tag="eblk",
                                           name="eblk")
                    edma.dma_start(out=pair_tile[:, :span],
                                   in_=edges[:, s * sb_e:s * sb_e + span])
                eblk = pair_tile[:, (s % 2) * sb_e:(s % 2 + 1) * sb_e]
            else:
                eblk = epool.tile([P, sb_e], f16, tag="eblk", name="eblk")
                edma.dma_start(out=eblk[:],
                               in_=edges[:, s * sb_e:(s + 1) * sb_e])
        if only == "dma":
            continue

        if only == "dve":
            for c in range(SB * KREM):
                oh = ohpool.tile([P, 128], f16, tag="oh", name="oh")
                nc.vector.tensor_scalar(
                    out=oh[:], in0=iota_s[:],
                    scalar1=colf_s[:, s * SB * KREM + c:
                                   s * SB * KREM + c + 1],
                    scalar2=None, op0=Alu.is_equal)
            continue

        pag = psag.tile([P, SB * 128], f32, tag="ag", name="ag",
                        bufs=TUNE["agbufs"])
        pqs = []
        if TUNE["s1_first"]:
            for b4 in range(SB):
                ebase = (b4 * KTOT) * 128
                pq = pspq.tile([P, 512], f32, tag="pq", name="pq",
                               bufs=TUNE["pqbufs"])
                pqs.append(pq)
                for t in range(KMAIN):
                    i, q = t % 4, t // 4
                    nc.tensor.matmul(
                        out=pq[32 * i:32 * i + 32, 128 * q:128 * q + 128],
                        lhsT=R[:],
                        rhs=eblk[:, ebase + t * 128:ebase + (t + 1) * 128],
                        tile_position=(0, 32 * i), start=True, stop=True)
        for b4 in range(SB):
            ebase = (b4 * KTOT) * 128
            if TUNE["s1_first"]:
                pq = pqs[b4]
            else:
                # stage 1: 16 main tiles -> [128,512] groups (col-tiled 4x)
                pq = pspq.tile([P, 512], f32, tag="pq", name="pq",
                               bufs=TUNE["pqbufs"])
                for t in range(KMAIN):
                    i, q = t % 4, t // 4
                    nc.tensor.matmul(
                        out=pq[32 * i:32 * i + 32, 128 * q:128 * q + 128],
                        lhsT=R[:],
                        rhs=eblk[:, ebase + t * 128:ebase + (t + 1) * 128],
                        tile_position=(0, 32 * i), start=True, stop=True)
            if only == "s1":
                continue
            grp = spool.tile([P, 512], f16, tag="grp", name="grp",
                             bufs=TUNE["grpbufs"])
            geng = nc.vector if b4 < TUNE["grp_dve"] else nc.scalar
            if geng is nc.vector:
                nc.vector.tensor_copy(grp[:], pq[:])
            else:
                nc.scalar.copy(grp[:], pq[:])
            # stage 2: 2 remainder one-hot matmuls open the accumulation,
            # then 4 constant-scatter matmuls close it (no DVE deps late).
            cbase = s * SB * KREM + b4 * KREM
            if TUNE["rem_first"]:
                for r in range(KREM):
                    oh = ohpool.tile([P, 128], f16, tag="oh", name="oh")
                    nc.vector.tensor_scalar(
                        out=oh[:], in0=iota_s[:],
                        scalar1=colf_s[:, cbase + r:cbase + r + 1],
                        scalar2=None, op0=Alu.is_equal)
                    nc.tensor.matmul(
                        out=pag[:, b4 * 128:(b4 + 1) * 128],
                        lhsT=eblk[:, ebase + (KMAIN + r) * 128:
                                  ebase + (KMAIN + r + 1) * 128],
                        rhs=oh[:], start=(r == 0), stop=False)
                for q in range(SB):
                    nc.tensor.matmul(out=pag[:, b4 * 128:(b4 + 1) * 128],
                                     lhsT=grp[:, q * 128:(q + 1) * 128],
                                     rhs=S[:, q, :], start=False,
                                     stop=(q == SB - 1))
            else:
                for q in range(SB):
                    nc.tensor.matmul(out=pag[:, b4 * 128:(b4 + 1) * 128],
                                     lhsT=grp[:, q * 128:(q + 1) * 128],
                                     rhs=S[:, q, :], start=(q == 0),
                                     stop=False)
                for r in range(KREM):
                    oh = ohpool.tile([P, 128], f16, tag="oh", name="oh")
                    nc.vector.tensor_scalar(
                        out=oh[:], in0=iota_s[:],
                        scalar1=colf_s[:, cbase + r:cbase + r + 1],
                        scalar2=None, op0=Alu.is_equal)
                    nc.tensor.matmul(
                        out=pag[:, b4 * 128:(b4 + 1) * 128],
                        lhsT=eblk[:, ebase + (KMAIN + r) * 128:
                                  ebase + (KMAIN + r + 1) * 128],
                        rhs=oh[:], start=False, stop=(r == KREM - 1))
        if only in ("agg", "s1"):
            continue

        aggrT = spool.tile([P, SB * 128], f16, tag="aggrT", name="aggrT")
        if TUNE["aggrT_dve"]:
            nc.vector.tensor_copy(aggrT[:], pag[:])
        else:
            nc.scalar.copy(aggrT[:], pag[:])

        # MLP (fp16 weights, fp32 PSUM accumulate)
        ph1 = psmlp.tile([P, SB * 128], f32, tag="mlp", name="mlp")
        nc.tensor.matmul(out=ph1[:], lhsT=consts["w0a"][:],
                         rhs=natT_s[:, s * SB * 128:(s + 1) * SB * 128],
                         start=True, stop=False)
        nc.tensor.matmul(out=ph1[:], lhsT=consts["w0b"][:], rhs=aggrT[:],
                         start=False, stop=True)
        h1 = spool.tile([P, SB * 128], f16, tag="h1", name="h1")
        nc.scalar.activation(h1[:], ph1[:], Act.Relu, bias=consts["b0"][:])

        ph2 = psmlp.tile([P, SB * 128], f32, tag="mlp", name="mlp")
        nc.tensor.matmul(out=ph2[:], lhsT=consts["w1"][:], rhs=h1[:],
                         start=True, stop=True)
        h2 = spool.tile([P, SB * 128], f16, tag="h2", name="h2")
        nc.scalar.activation(h2[:], ph2[:], Act.Relu, bias=consts["b1"][:])

        ph3 = psmlp.tile([P, SB * 128], f32, tag="mlp", name="mlp")
        nc.tensor.matmul(out=ph3[:], lhsT=consts["w2"][:], rhs=h2[:],
                         start=True, stop=True)
        h3T = spool.tile([P, SB * 128], f16, tag="h3T", name="h3T")
        nc.scalar.activation(h3T[:], ph3[:], Act.Identity,
                             bias=consts["b2"][:])

        pyt = psag.tile([P, SB, 128], f32, tag="py", name="py", bufs=2)
        for b4 in range(SB):
            nc.tensor.matmul(out=pyt[:, b4, :],
                             lhsT=h3T[:, b4 * 128:(b4 + 1) * 128],
                             rhs=consts["ident"][:], start=True, stop=True)
        # node-major LayerNorm stats straight off the PSUM transpose
        stats = spool.tile([P, SB, 6], f32, tag="stats", name="stats")
        for b4 in range(SB):
            nc.vector.bn_stats(stats[:, b4, :], pyt[:, b4, :])
        mv = spool.tile([P, SB, 2], f32, tag="mv", name="mv")
        for b4 in range(SB):
            nc.vector.bn_aggr(mv[:, b4, :], stats[:, b4, :])
        std = spool.tile([P, SB], f32, tag="std", name="std")
        nc.scalar.activation(std[:], mv[:, :, 1], Act.Sqrt, bias=epst[:])
        rstd = spool.tile([P, SB], f32, tag="rstd", name="rstd")
        nc.vector.reciprocal(rstd[:], std[:])
        xn = spool.tile([P, SB, 128], f16, tag="xn", name="xn")
        for b4 in range(SB):
            nc.vector.tensor_scalar(
                out=xn[:, b4, :], in0=pyt[:, b4, :],
                scalar1=mv[:, b4, 0:1], scalar2=rstd[:, b4:b4 + 1],
                op0=Alu.subtract, op1=Alu.mult)
        geng = nc.gpsimd if TUNE["gam_engine"] == "gp" else nc.vector
        beng = nc.gpsimd if TUNE["beta_engine"] == "gp" else nc.vector
        yg = spool.tile([P, SB, 128], f16, tag="yg", name="yg")
        geng.tensor_tensor(out=yg[:], in0=xn[:], in1=consts["gam"][:],
                           op=Alu.mult)
        yo = spool.tile([P, SB, 128], f16, tag="yo", name="yo")
        beng.tensor_tensor(out=yo[:], in0=yg[:], in1=consts["bet"][:],
                           op=Alu.add)
        odma.dma_start(out=out[s], in_=yo[:])


def _prepare_shards(node_attr, edge_attr, col):
    """Fixed 16-slot-per-node main region + LPT on overflow for remainder."""
    import heapq

    deg = np.bincount(col, minlength=NUM_NODES).astype(np.int64)
    over = np.maximum(deg - CAP, 0)
    order_nodes = np.argsort(-over, kind="stable")
    heap = [(0, 0, b) for b in range(TOTAL_BLOCKS)]
    heapq.heapify(heap)
    block_nodes = [[] for _ in range(TOTAL_BLOCKS)]
    for nd in order_nodes:
        d = int(over[nd])
        s, cnt, b = heapq.heappop(heap)
        block_nodes[b].append(int(nd))
        if cnt + 1 < BLK:
            heapq.heappush(heap, (s + d, cnt + 1, b))
    rem_max = max(sum(int(over[nd]) for nd in bn) for bn in block_nodes)
    assert rem_max <= KREM * 128, rem_max

    pos = np.full(NUM_NODES, -1, dtype=np.int64)      # old -> new node id
    natp = np.zeros((TOTAL_BLOCKS * BLK, D), np.float16)
    for b, bn in enumerate(block_nodes):
        ids = np.asarray(bn, dtype=np.int64)
        pos[ids] = b * BLK + np.arange(len(ids))
        natp[b * BLK:b * BLK + len(ids)] = node_attr[ids].astype(np.float16)
    assert (pos >= 0).all()

    # per-edge slot assignment
    order = np.argsort(col, kind="stable")           # edges grouped per node
    cs = col[order]
    within = np.arange(col.shape[0], dtype=np.int64)
    starts = np.zeros(NUM_NODES + 1, np.int64)
    starts[1:] = np.cumsum(deg)
    within = within - starts[cs]                     # rank within node
    npos = pos[cs]
    blk = npos >> 7
    loc = npos & 127

    main_mask = within < CAP
    slot = np.empty(col.shape[0], dtype=np.int64)
    slot[main_mask] = (blk[main_mask] * KTOT * 128 + loc[main_mask] * CAP +
                       within[main_mask])
    # overflow edges: sequential within their block's remainder region
    om = ~main_mask
    oblk = blk[om]
    oord = np.argsort(oblk, kind="stable")
    ocnt = np.bincount(oblk, minlength=TOTAL_BLOCKS)
    ostart = np.zeros(TOTAL_BLOCKS + 1, np.int64)
    ostart[1:] = np.cumsum(ocnt)
    opos_in_blk = np.arange(om.sum(), dtype=np.int64) - ostart[oblk[oord]]
    oslot = np.empty(om.sum(), dtype=np.int64)
    oslot[oord] = (oblk[oord] * KTOT * 128 + KMAIN * 128 + opos_in_blk)
    slot[om] = oslot

    ea16 = edge_attr.astype(np.float16)
    slots_per_core = BLOCKS_PER_CORE * KTOT * 128
    edges_by_core = []
    colf_by_core = []
    natT_by_core = []
    blk_of = slot // (KTOT * 128)
    off_of = slot % (KTOT * 128)
    loc_f = loc.astype(np.float32)
    for c in range(N_CORES):
        sel = (blk_of >= c * BLOCKS_PER_CORE) & \
              (blk_of < (c + 1) * BLOCKS_PER_CORE)
        lblk = blk_of[sel] - c * BLOCKS_PER_CORE
        lslot = lblk * (KTOT * 128) + off_of[sel]
        ebuf = np.zeros((slots_per_core, D), np.float16)
        ebuf[lslot] = ea16[order[sel]]
        earr = np.ascontiguousarray(
            ebuf.reshape(BLOCKS_PER_CORE * KTOT, 128, D)
            .transpose(1, 0, 2).reshape(P, slots_per_core))
        edges_by_core.append(earr)
        cf = np.full((BLOCKS_PER_CORE, KREM, 128), -1.0, np.float32)
        rm = off_of[sel] >= KMAIN * 128
        roff = off_of[sel][rm] - KMAIN * 128
        cf[lblk[rm], roff // 128, roff % 128] = loc_f[sel][rm]
        carr = np.ascontiguousarray(
            cf.reshape(BLOCKS_PER_CORE * KREM, 128).T)
        colf_by_core.append(carr)
        natT_by_core.append(np.ascontiguousarray(
            natp[c * NODES_PER_CORE:(c + 1) * NODES_PER_CORE].T))
    kb = (KTOT,) * BLOCKS_PER_CORE
    return kb, edges_by_core, colf_by_core, natT_by_core, pos


def kernel(node_attr, edge_attr, edge_index, W0, b0, W1, b1, W2, b2,
           ln_g, ln_b):
    from concourse import bass_utils

    node_attr = np.ascontiguousarray(np.asarray(node_attr, dtype=np.float32))
    edge_attr = np.ascontiguousarray(np.asarray(edge_attr, dtype=np.float32))
    col = np.asarray(edge_index)[1].astype(np.int64)
    W0 = np.asarray(W0, dtype=np.float32)
    W1 = np.asarray(W1, dtype=np.float32)
    W2 = np.asarray(W2, dtype=np.float32)
    b0v = np.asarray(b0, np.float32).reshape(128, 1).copy()
    b1v = np.asarray(b1, np.float32).reshape(128, 1).copy()
    b2v = np.asarray(b2, np.float32).reshape(128, 1).copy()
    gam = np.ascontiguousarray(
        np.broadcast_to(np.asarray(ln_g, np.float32).reshape(1, 1, 128),
                        (128, SB, 128)).astype(np.float16))
    bet = np.ascontiguousarray(
        np.broadcast_to(np.asarray(ln_b, np.float32).reshape(1, 1, 128),
                        (128, SB, 128)).astype(np.float16))

    kb, edges_by_core, colf_by_core, natT_by_core, pos = _prepare_shards(
        node_attr, edge_attr, col)

    iota_rep = np.ascontiguousarray(
        np.broadcast_to(np.arange(128, dtype=np.float16), (P, 128)))
    rmat = np.zeros((128, 32), np.float16)
    rmat[np.arange(128), np.arange(128) // G] = 1.0
    # S_q[p, j] = 1 iff node j owns group at partition p of quad-column q:
    #   j = 32 q + 8 (p//32) + (p%32)//4
    smat = np.zeros((128, SB, 128), np.float16)
    pidx = np.arange(128)
    for q in range(SB):
        smat[pidx, q, 32 * q + 8 * (pidx // 32) + (pidx % 32) // 4] = 1.0
    ident = np.eye(128, dtype=np.float16)

    if kb not in _nc_cache:
        _nc_cache[kb] = _build_nc(kb)
    nc = _nc_cache[kb]

    shared = {"iota": iota_rep, "rmat": rmat, "smat": smat, "ident": ident,
              "w0a": np.ascontiguousarray(W0[:128].astype(np.float16)),
              "w0b": np.ascontiguousarray(W0[128:].astype(np.float16)),
              "w1": np.ascontiguousarray(W1.astype(np.float16)),
              "w2": np.ascontiguousarray(W2.astype(np.float16)),
              "gam": gam, "bet": bet, "b0": b0v, "b1": b1v, "b2": b2v}
    in_maps = []
    for c in range(N_CORES):
        m = {"edges": edges_by_core[c], "colf32": colf_by_core[c],
             "natT": natT_by_core[c]}
        m.update(shared)
        in_maps.append(m)

    res = bass_utils.run_bass_kernel_spmd(nc, in_maps,
                                          core_ids=list(range(N_CORES)))
    last_run_info["results"] = res
    last_run_info["nc"] = nc
    last_run_info["in_maps"] = in_maps
    last_run_info["kb"] = kb
    last_run_info["pos"] = pos

    rows = np.concatenate(
        [res.results[c]["out"].reshape(SBLOCKS, P, SB, 128)
         .transpose(0, 2, 1, 3).reshape(NODES_PER_CORE, D)
         for c in range(N_CORES)], axis=0)
    return rows[pos].astype(np.float32)


if __name__ == "__main__":
    pass
